# revision 29
# baseline (speedup 1.0000x reference)
"""GQA causal attention kernel for 8 Trainium2 NeuronCores (v3).

Sharding: data-parallel over batch (2) x tensor-parallel over head groups (4).
Core c handles batch b = c // 4 and head group g = c % 4 (query heads
4g..4g+3, KV head g, Wo rows 512g..512(g+1)).  Each core computes a full
[N, DIM] partial of the output projection (bf16); the host sums the 4
partials per batch in fp32.

v3 changes over v2:
  - DMA batching: one consts tile (mask|identity), one wkv tile (1 DMA),
    one wq tile (2 DMAs), one wo tile (1 DMA), x t1-3 batched per d-chunk
    (16 DMAs of [128,1536]), output stores batched 4 quarters -> 1 DMA.
    ~55 DMAs total (was 153) to cut HWDGE descriptor serialization.
  - V^T->V transposes are deferred into the following projection strip so
    the PE does not stall on the DVE copy latency.
  - Softmax row sums come out of the PV matmul itself via a ones column
    appended to V chunks ([V_j | 1], 129 cols); P^T chunks are stationary.
  - Normalization is a per-partition tensor_scalar multiply on O natural;
    O^T for the output projection is produced by PE transposes.
"""

import os
import numpy as np

B, N, DIM = 2, 2048, 2048
H, KVH, HD = 16, 4, 128
HQ = H // KVH          # query heads per core
SCALE = float(HD) ** -0.5
NT = N // 128          # 16 seq tiles
DC = DIM // 128        # 16 contraction chunks
NB = 4                 # q bands of 512
BW = N // NB           # 512 band width
VW = 129               # V chunk width incl. ones column

_cache = {}


def _build(reps=1):
    import concourse.bass as bass
    import concourse.bacc as bacc
    import concourse.tile as tile
    import concourse.mybir as mybir

    f32 = mybir.dt.float32
    bf16 = mybir.dt.bfloat16
    EXP = mybir.ActivationFunctionType.Exp

    nc = bacc.Bacc("TRN2", target_bir_lowering=False, debug=False)

    xT = nc.dram_tensor("xT", [DIM, N], bf16, kind="ExternalInput")
    wq = nc.dram_tensor("wq", [DIM, HQ * HD], bf16, kind="ExternalInput")
    wkv = nc.dram_tensor("wkv", [DIM, 2 * HD], bf16, kind="ExternalInput")
    wo = nc.dram_tensor("wo", [HQ * HD, DIM], bf16, kind="ExternalInput")
    consts = nc.dram_tensor("consts", [128, 256], bf16, kind="ExternalInput")
    out = nc.dram_tensor("out", [N, DIM], bf16, kind="ExternalOutput")

    with tile.TileContext(nc) as tc:
        from contextlib import ExitStack

        with ExitStack() as ctx:
            resident = ctx.enter_context(tc.tile_pool(name="resident", bufs=1))

            # --- resident tiles ---
            qt = resident.tile([128, HQ * N], bf16)        # Q^T all heads
            kt = resident.tile([128, N], bf16)             # K^T
            vone = resident.tile([128, NT * VW], bf16)     # [V_j | 1] chunks
            consts_sb = resident.tile([128, 256], bf16)
            m01_sb = consts_sb[:, 0:128]
            id_sb = consts_sb[:, 128:256]
            wo_sb = resident.tile([128, HQ * DIM], bf16, tag="wo", name="wo_sb")
            ot_sb2 = [resident.tile([128, HQ * BW], bf16, tag=f"ot_sb{i}",
                                    name=f"ot_sb{i}") for i in range(2)]

            # ---------------- Phase 1: projections ----------------
            xth_pool = ctx.enter_context(tc.tile_pool(name="xth", bufs=1))
            wqh_pool = ctx.enter_context(tc.tile_pool(name="wqh", bufs=1))
            for rep in range(reps):
              R = f"_r{rep}" if rep else ""
              with nc.named_scope("proj"):
                with (
                    tc.tile_pool(name="wkv" + R, bufs=1) as wkv_pool,
                    tc.tile_pool(name="pp" + R, bufs=8, space="PSUM") as pp,
                    tc.tile_pool(name="vt" + R, bufs=1) as vt_pool,
                ):
                    # PE warmup while input DMAs land: keeps the tensor
                    # engine p-state ramp going before real work. Streams a
                    # memset tile (no DMA dependency); the sink DMA at the
                    # end keeps the chain from being dead-code eliminated.
                    # The memset also provides the ones columns of vone.
                    if rep == 0:
                        nc.vector.memset(vone[:, 0:512], 1.0)
                        warm = pp.tile([128, 512], f32, tag="acc")
                        for _ in range(14):
                            nc.tensor.matmul(warm[:], vone[:, 0:128],
                                             vone[:, 0:512])
                        nc.vector.memset(vone[:, 512:], 1.0)
                        wsink = vt_pool.tile([128, 512], f32, tag="wsink")

                    wkv_sb = wkv_pool.tile([128, DC * 2 * HD], bf16, tag="wkv")
                    vtmp = vt_pool.tile([128, N], bf16)    # V^T staging

                    # x^T lives in ONE d-major tile so DMAs batch 4 d-chunks
                    # per descriptor: xq[:, d*N + t*512 : ...] is chunk (d,t).
                    xq = xth_pool.tile([128, DC * N], bf16, tag="xq",
                                       name=f"xq{R}")

                    def xdma(eng, g, c0, c1):
                        dst = xq[:, g * 4 * N:(g + 1) * 4 * N].rearrange(
                            "p (d c) -> p d c", d=4)[:, :, c0:c1]
                        src = xT.ap()[g * 512:(g + 1) * 512, c0:c1].rearrange(
                            "(d p) c -> p d c", p=128)
                        eng.dma_start(dst, src)

                    # One big queue (SP) carries everything size-ordered for
                    # the pipe: x t0, wq, x t1-3, wo.  ACT only issues the
                    # small consts + wkv ahead of it (the two queues race,
                    # so keeping the big transfers on one queue preserves
                    # priority order on the DMA pipe).
                    nc.scalar.dma_start(consts_sb[:], consts.ap())
                    nc.sync.dma_start(
                        wkv_sb[:].rearrange("p (d c) -> p d c", d=DC),
                        wkv.ap().rearrange("(d p) c -> p d c", p=128))
                    for g in range(4):
                        xdma(nc.sync, g, 0, 512)
                    wq_sb = wqh_pool.tile([128, DC * HQ * HD], bf16, tag="wq",
                                          name=f"wq_sb{R}")
                    for half in range(2):
                        rows = slice(half * 8 * 128, (half + 1) * 8 * 128)
                        nc.sync.dma_start(
                            wq_sb[:, half * 8 * 512:(half + 1) * 8 * 512]
                            .rearrange("p (d c) -> p d c", d=8),
                            wq.ap()[rows].rearrange("(d p) c -> p d c", p=128))
                    for g in range(4):
                        xdma(nc.sync, g, 512, N)

                    # Wo last (needed only at outproj).
                    nc.sync.dma_start(
                        wo_sb[:].rearrange("p (h c) -> p h c", h=HQ),
                        wo.ap().rearrange("(h p) c -> p h c", p=128))

                    if rep == 0:
                        # warmup sink (ACT queue, after all input DMAs)
                        nc.vector.tensor_copy(wsink[:], warm[:])
                        sink_dram = nc.dram_tensor("warm_sink", [128, 512],
                                                   f32, kind="Internal")
                        nc.scalar.dma_start(sink_dram.ap(), wsink[:])

                    # t-block waves: K_t, V_t, then Q_t for each head.
                    # Q for t-blocks 2,3 is deferred into band 0's pipeline
                    # bubbles (emitted via the op pool during attention).
                    # V^T -> V transposes are deferred into the next strip.
                    pending_tp = []

                    def flush_tp():
                        for t in pending_tp:
                            for jj in range(4):
                                j = 4 * t + jj
                                tp = pp.tile([128, 128], bf16, tag="acc")
                                nc.tensor.transpose(
                                    tp[:], vtmp[:, j * 128:(j + 1) * 128],
                                    id_sb)
                                nc.vector.tensor_copy(
                                    vone[:, j * VW: j * VW + 128], tp[:])
                        pending_tp.clear()

                    # strip order: K/V t0, Q t0 (4 heads), K/V t1..t3, then
                    # Q t1 — so the PE always has strip work after the last
                    # V strip while its transposes drain, and band-0
                    # attention follows Q t1 without a phase-boundary stall.
                    # Q t2/t3 are deferred into band-0/1 bubbles.
                    strips = [(0, 0), (0, 1), (0, 2), (0, 3), (0, 4), (0, 5),
                              (1, 0), (1, 1), (2, 0), (2, 1), (3, 0), (3, 1),
                              (1, 2), (1, 3), (1, 4), (1, 5)]
                    for t, s in strips:
                        acc = pp.tile([128, 512], f32, tag="acc")
                        for d in range(DC):
                            if s == 0:
                                lhs = wkv_sb[:, d * 256:d * 256 + HD]
                            elif s == 1:
                                lhs = wkv_sb[:, d * 256 + HD:(d + 1) * 256]
                            else:
                                h = s - 2
                                lhs = wq_sb[:, d * 512 + h * HD:
                                            d * 512 + (h + 1) * HD]
                            nc.tensor.matmul(
                                acc[:], lhs,
                                xq[:, d * N + t * 512:
                                   d * N + (t + 1) * 512],
                                start=(d == 0), stop=(d == DC - 1))
                            if d == 8 and pending_tp:
                                flush_tp()
                        if s == 0:
                            dst = kt[:, t * 512:(t + 1) * 512]
                        elif s == 1:
                            dst = vtmp[:, t * 512:(t + 1) * 512]
                        else:
                            h = s - 2
                            dst = qt[:, h * N + t * 512: h * N + (t + 1) * 512]
                        nc.vector.tensor_copy(dst, acc[:])
                        if s == 1:
                            pending_tp.append(t)
                    flush_tp()

              # ---------------- Phase 2: attention + out-projection ----------------
              with nc.named_scope("attn"):
                with (
                    tc.tile_pool(name="pt" + R, bufs=4) as pt_pool,
                    tc.tile_pool(name="onorm" + R, bufs=2) as on_pool,
                    tc.tile_pool(name="rr" + R, bufs=2) as rr_pool,
                    tc.tile_pool(name="stage" + R, bufs=3) as stage_pool,
                    tc.tile_pool(name="st" + R, bufs=2, space="PSUM") as st_pool,
                    tc.tile_pool(name="oz" + R, bufs=2, space="PSUM") as oz_pool,
                    tc.tile_pool(name="op" + R, bufs=2, space="PSUM") as op_pool,
                ):
                    COPY = mybir.ActivationFunctionType.Copy
                    stage_t = {}

                    def outproj_quarter(Ib, t, dt):
                        """One [128 q, 512 dim] chunk of band Ib's output
                        projection: 4 accumulating matmuls + copy; the
                        four dt quarters share one stage tile, DMA'd as a
                        single [128, 2048] store on dt == 3."""
                        src = ot_sb2[Ib % 2]
                        key = (Ib, t)
                        if key not in stage_t:
                            stage_t[key] = stage_pool.tile(
                                [128, DIM], bf16, tag="stg",
                                name=f"stg{Ib}_{t}{R}")
                        stgq = stage_t[key]
                        opp = op_pool.tile([128, 512], f32, tag="op",
                                           name=f"op{Ib}_{t}_{dt}{R}")
                        for hh in range(HQ):
                            nc.tensor.matmul(
                                opp[:],
                                src[:, hh * BW + t * 128: hh * BW + (t + 1) * 128],
                                wo_sb[:, hh * DIM + dt * 512:
                                      hh * DIM + (dt + 1) * 512],
                                start=(hh == 0), stop=(hh == HQ - 1))
                        nc.vector.tensor_copy(
                            stgq[:, dt * 512:(dt + 1) * 512], opp[:])
                        if Ib == NB - 1:
                            # final band: store per quarter so the kernel
                            # tail only drains a [128,512] DMA
                            nc.sync.dma_start(
                                out.ap()[Ib * BW + t * 128:
                                         Ib * BW + (t + 1) * 128,
                                         dt * 512:(dt + 1) * 512],
                                stgq[:, dt * 512:(dt + 1) * 512])
                            if dt == 3:
                                del stage_t[key]
                        elif dt == 3:
                            nc.sync.dma_start(
                                out.ap()[Ib * BW + t * 128:
                                         Ib * BW + (t + 1) * 128, :],
                                stgq[:])
                            del stage_t[key]

                    def q_acc(t, hq):
                        """Deferred Q projection for one (t-block, head):
                        16 accumulating matmuls through the op pool."""
                        acc = op_pool.tile([128, 512], f32, tag="op",
                                           name=f"qacc{t}_{hq}{R}")
                        for d in range(DC):
                            nc.tensor.matmul(
                                acc[:],
                                wq_sb[:, d * 512 + hq * HD:
                                      d * 512 + (hq + 1) * HD],
                                xq[:, d * N + t * 512:
                                   d * N + (t + 1) * 512],
                                start=(d == 0), stop=(d == DC - 1))
                        nc.vector.tensor_copy(
                            qt[:, hq * N + t * 512: hq * N + (t + 1) * 512],
                            acc[:])

                    from collections import deque

                    qpending = deque((t, hq) for t in (2, 3)
                                     for hq in range(HQ))
                    pending = deque()
                    for I in range(NB):
                        jmax = 4 * I + 3
                        ot_sb = ot_sb2[I % 2]
                        # previous band's output projection, interleaved into
                        # this band's S-to-PV pipeline bubbles
                        if I > 0:
                            pending.extend(
                                (I - 1, t, dt) for t in range(4) for dt in range(4))
                        if I == 2:
                            while qpending:
                                q_acc(*qpending.popleft())
                        slot = 0
                        for h in range(HQ):
                            # two banks: c0,c1 in ozt[0] (cols 0:129,129:258),
                            # c2,c3 in ozt[1].
                            ozt = [oz_pool.tile([128, 512], f32, tag="oz",
                                                name=f"oz{I}_{h}_0{R}"),
                                   oz_pool.tile([128, 512], f32, tag="oz",
                                                name=f"oz{I}_{h}_1{R}")]
                            # PSUM accumulation groups are zero-region (bank)
                            # granular: exactly one start (bank's first
                            # matmul, zeroes the whole bank) and one stop
                            # (bank's last matmul) per bank per head.
                            oz_started = [False, False]
                            rrt = rr_pool.tile([128, 4], f32, tag="rr")
                            ont = on_pool.tile([128, BW], bf16, tag="on")

                            def norm_bank(bank):
                                # 1/Z then per-partition scale + XBAR
                                # transpose for this bank's two chunks;
                                # emitted as soon as the bank's accumulation
                                # group closes so the bank recycles a
                                # pipeline step earlier.
                                for c in (2 * bank, 2 * bank + 1):
                                    lane = c % 2
                                    nc.vector.reciprocal_approx_fast(
                                        rrt[:, c:c + 1],
                                        ozt[bank][:, lane * 256 + 128:
                                                  lane * 256 + VW])
                                    nc.vector.tensor_scalar_mul(
                                        ont[:, c * 128:(c + 1) * 128],
                                        ozt[bank][:, lane * 256: lane * 256 + 128],
                                        rrt[:, c:c + 1])

                            for p in range((jmax + 1) // 2):
                                stp = st_pool.tile([128, 2 * BW], f32, tag="st")
                                for u in range(2):
                                    j = 2 * p + u
                                    o = j - 4 * I
                                    qlo = max(0, o) * 128
                                    nc.tensor.matmul(
                                        stp[:, u * BW + qlo:(u + 1) * BW],
                                        kt[:, j * 128:(j + 1) * 128],
                                        qt[:, h * N + I * BW + qlo:
                                           h * N + (I + 1) * BW])
                                ptp = pt_pool.tile([128, 2 * BW], bf16, tag="pt")
                                nc.scalar.activation(ptp[:], stp[:], EXP, scale=SCALE)
                                # fill the exp-latency bubble with a chunk of
                                # an earlier band's output projection, or a
                                # deferred Q projection during band 0
                                take = (I == 1 or
                                        (I == 2 and slot % 3 != 0) or
                                        (I == 3 and slot % 2 == 1))
                                if pending and take:
                                    outproj_quarter(*pending.popleft())
                                elif qpending:
                                    q_acc(*qpending.popleft())
                                slot += 1
                                for u in range(2):
                                    j = 2 * p + u
                                    o = j - 4 * I
                                    if o >= 0:
                                        qlo = o * 128
                                        # triangular boundary tile
                                        nc.vector.tensor_mul(
                                            ptp[:, u * BW + qlo: u * BW + qlo + 128],
                                            ptp[:, u * BW + qlo: u * BW + qlo + 128],
                                            m01_sb)
                                    # diagonal steps: c descending so the
                                    # masked chunk (c == o) lands last and its
                                    # DVE dependency overlaps the others;
                                    # otherwise ascending so the next head's
                                    # first chunks hit the earliest-freed bank
                                    order = (range(3, max(0, o) - 1, -1)
                                             if o >= 0 else range(4))
                                    for c in order:
                                        bank, lane = divmod(c, 2)
                                        first = not oz_started[bank]
                                        oz_started[bank] = True
                                        last = (c == 2 * bank + 1
                                                and j == 4 * I + c)
                                        # O natural [q, d] + Z in col 128
                                        nc.tensor.matmul(
                                            ozt[bank][:, lane * 256: lane * 256 + VW],
                                            ptp[:, u * BW + c * 128:
                                                u * BW + (c + 1) * 128],
                                            vone[:, j * VW:(j + 1) * VW],
                                            start=first, stop=last)
                                        if last:
                                            norm_bank(bank)
                            # O^T for the whole head: 4 PE transposes into
                            # one op-pool tile, one DVE copy out
                            tpo = op_pool.tile([128, BW], bf16, tag="op",
                                               name=f"tpo{I}_{h}{R}")
                            for c in range(4):
                                nc.tensor.transpose(
                                    tpo[:, c * 128:(c + 1) * 128],
                                    ont[:, c * 128:(c + 1) * 128], id_sb)
                            nc.vector.tensor_copy(
                                ot_sb[:, h * BW:(h + 1) * BW], tpo[:])
                    # leftovers, then the last band's projection
                    while pending:
                        outproj_quarter(*pending.popleft())
                    for t in range(4):
                        for dt in range(4):
                            outproj_quarter(NB - 1, t, dt)

    nc.compile()
    return nc


def _get_nc(reps=1):
    key = f"nc{reps}"
    if key not in _cache:
        _cache[key] = _build(reps=reps)
    return _cache[key]


def _host_inputs(x, Wq, Wk, Wv, Wo):
    import ml_dtypes
    bf = ml_dtypes.bfloat16
    x = np.asarray(x, dtype=np.float32)
    Wq = np.asarray(Wq, dtype=bf)
    Wk = np.asarray(Wk, dtype=bf)
    Wv = np.asarray(Wv, dtype=bf)
    Wo = np.asarray(Wo, dtype=bf)

    kk = np.arange(128)[:, None]
    qq = np.arange(128)[None, :]
    m01 = (qq >= kk).astype(bf)
    consts = np.concatenate([m01, np.eye(128, dtype=bf)], axis=1)
    consts = np.ascontiguousarray(consts)

    xTb = [np.ascontiguousarray(x[b].T).astype(bf) for b in range(B)]
    in_maps = []
    for c in range(8):
        b, g = c // 4, c % 4
        wkv = np.concatenate(
            [Wk[:, g * HD:(g + 1) * HD], Wv[:, g * HD:(g + 1) * HD]], axis=1)
        in_maps.append({
            "xT": xTb[b],
            "wq": np.ascontiguousarray(Wq[:, g * 512:(g + 1) * 512]),
            "wkv": np.ascontiguousarray(wkv),
            "wo": np.ascontiguousarray(Wo[g * 512:(g + 1) * 512, :]),
            "consts": consts,
        })
    return in_maps


def run(x, mask, Wq, Wk, Wv, Wo, trace=False, trace_cores=None):
    from concourse.bass_utils import run_bass_kernel_spmd

    nc = _get_nc()
    in_maps = _host_inputs(x, Wq, Wk, Wv, Wo)
    res = run_bass_kernel_spmd(
        nc, in_maps, core_ids=list(range(8)), trace=trace,
        trace_cores=trace_cores)
    full = np.empty((B, N, DIM), dtype=np.float32)
    for b in range(B):
        acc = res.results[b * 4 + 0]["out"].astype(np.float32)
        for g in range(1, 4):
            acc = acc + res.results[b * 4 + g]["out"].astype(np.float32)
        full[b] = acc
    return full, res


def kernel(x, mask, Wq, Wk, Wv, Wo):
    out, _ = run(x, mask, Wq, Wk, Wv, Wo, trace=False)
    return out
